# revision 59
# baseline (speedup 1.0000x reference)
"""MidGCN forward on 8 Trainium2 NeuronCores (Bass/Tile, SPMD row-sharding).

Math (alpha = 0.5), with P(y) = A @ (Dc y), Dr/Dc = diag of rsqrt row/col
sums:
  adj_f @ y = 0.5*y - Dr*(0.5*P(y) + P(Dr*P(y)))
  h   = relu(adj_f @ (x @ W1))
  out = log_softmax(adj_f @ (h @ W2) + b2)

Implementation notes:
- The adj slab is shipped host-transposed in fp8(e4m3) ([8192, 1024] per
  core) and every P() application runs DoubleRow fp8 matmuls (K=256 per
  instruction).  Moving operands (the gathered activations) are pre-scaled
  by sigma*dcol (sigma a power of two chosen to land the values in e4m3's
  normal range) and quantized to fp8 *before* each AllGather, so the
  gathered tiles feed the PE directly.  The 0.5*y identity terms and all
  epilogue math stay in fp32 psum.
- Dr/Dc come from the host (one pass over adj there), so pass 1 is gated
  only by the slab DMA itself: the slab arrives as 32 pair-tiles and the
  pass-1 accumulation pipelines behind the load.
- Every accumulation group owns a full PSUM bank (hw start_tensor_calc
  zeroes bank-granular regions; two groups in one bank corrupt each
  other).
- The ACT engine holds the natural_log_exp_and_others table for the whole
  kernel (it also contains copy and relu), so scaled psum->SBUF copies and
  the final 2-class log-softmax (exp+ln) never reload tables.
- The zt/zv/zu gathers are split into two half-collectives pipelined a
  few microseconds apart, so the store->AllGather->reload latency of the
  first half hides under the second half's epilogue compute; the next
  pass consumes first-half pairs first.
- Latency-critical gather-chain DMAs issue from whichever engine queue
  is idle at that point (ACT early, SP late, gpsimd for bulk reloads) so
  they never wait behind the slab stream or each other.
- Output: each core computes its own [1024, 2] rows; host concatenates.
"""

import numpy as np
import ml_dtypes

NCORE = 8
N = 8192
NF = 512
NH = 256
NC = 2
RPC = N // NCORE          # rows per core = 1024
KT = N // 128             # 64 contraction tiles
PT = KT // 2              # 32 DoubleRow pair tiles
MT = RPC // 128           # 8 output row tiles per core
FT = NF // 128            # 4 k-tiles for x @ W1

# power-of-two pre-quantization scales per P() application
SIG1, SIG2, SIG3, SIG4 = 16.0, 2048.0, 8.0, 16.0

_CACHE = {}


def _build(lite=False, sim=False):
    import concourse.bass as bass
    import concourse.mybir as mybir
    import concourse.tile as tile
    from concourse import bacc, masks
    from concourse.bass import ts

    BF = mybir.dt.bfloat16
    F32 = mybir.dt.float32
    F8 = mybir.dt.float8e4
    OP = mybir.AluOpType
    AF = mybir.ActivationFunctionType
    DR = mybir.MatmulPerfMode.DoubleRow

    nc = bacc.Bacc("TRN2", target_bir_lowering=False, debug=False,
                   num_devices=NCORE)

    adjq = nc.dram_tensor("adjq", [N, RPC], F8, kind="ExternalInput")
    xT = nc.dram_tensor("xT", [NF, RPC], BF, kind="ExternalInput")
    w1 = nc.dram_tensor("w1", [NF, NH], BF, kind="ExternalInput")
    w2h = nc.dram_tensor("w2h", [NH, NC], BF, kind="ExternalInput")
    b2 = nc.dram_tensor("b2", [1, NC], F32, kind="ExternalInput")
    vecs = nc.dram_tensor("vecs", [RPC, 8], F32, kind="ExternalInput")
    out = nc.dram_tensor("out", [RPC, NC], F32, kind="ExternalOutput")

    zs_in = nc.dram_tensor("zs_in", [RPC, NH], F8)
    zs_out = nc.dram_tensor("zs_out", [N, NH], F8, addr_space="Shared")
    RH = RPC // 2
    zt_inA = nc.dram_tensor("zt_inA", [RH, NH], F8)
    zt_outA = nc.dram_tensor("zt_outA", [N // 2, NH], F8,
                             addr_space="Shared")
    zt_inB = nc.dram_tensor("zt_inB", [RH, NH], F8)
    zt_outB = nc.dram_tensor("zt_outB", [N // 2, NH], F8,
                             addr_space="Shared")
    zv_inA = nc.dram_tensor("zv_inA", [RH, NC], F8)
    zv_outA = nc.dram_tensor("zv_outA", [N // 2, NC], F8,
                             addr_space="Shared")
    zv_inB = nc.dram_tensor("zv_inB", [RH, NC], F8)
    zv_outB = nc.dram_tensor("zv_outB", [N // 2, NC], F8,
                             addr_space="Shared")
    zu_inA = nc.dram_tensor("zu_inA", [RH, NC], F8)
    zu_outA = nc.dram_tensor("zu_outA", [N // 2, NC], F8,
                             addr_space="Shared")
    zu_inB = nc.dram_tensor("zu_inB", [RH, NC], F8)
    zu_outB = nc.dram_tensor("zu_outB", [N // 2, NC], F8,
                             addr_space="Shared")
    RG = [list(range(NCORE))]

    if lite:
        # I/O-identical null kernel: measures tunnel/dispatch overhead.
        with tile.TileContext(nc) as tc:
            with tc.tile_pool(name="p0", bufs=1) as p0:
                o = p0.tile([128, MT, NC], F32, tag="o")
                nc.vector.memset(o, 0.0)
                nc.sync.dma_start(
                    out=out[:].rearrange("(mt p) c -> p mt c", p=128), in_=o)
        nc.compile()
        return nc

    def gather(z_in, z_out, eng=None):
        """AllGather (real) or I/O-shaped local stub (sim timing build).

        The stub also writes one row into each remote RPC block so that
        chunked loads of z_out keep their RAW dependency on the gather in
        the sim build, matching the real collective's semantics."""
        blk = z_in.shape[0]
        if eng is None:
            eng = nc.scalar
        if sim:
            eng.dma_start(out=z_out[0:blk, :], in_=z_in[:])
            eng.dma_start(
                out=z_out[blk:, :].rearrange(
                    "(a b) c -> a b c", b=blk)[:, 0, :],
                in_=z_in[0:NCORE - 1, :])
        else:
            nc.gpsimd.collective_compute(
                "AllGather", OP.bypass, replica_groups=RG,
                ins=[z_in[:]], outs=[z_out[:]])

    with tile.TileContext(nc) as tc:
        from contextlib import ExitStack
        with ExitStack() as ctx:
            p_adj = ctx.enter_context(tc.tile_pool(name="p_adj", bufs=PT))
            p_one = ctx.enter_context(tc.tile_pool(name="p_one", bufs=1))
            p_rot = ctx.enter_context(tc.tile_pool(name="p_rot", bufs=2))

            # ---------- persistent SBUF ----------
            xT_sb = p_one.tile([128, FT, RPC], BF, tag="xT")
            w1_sb = p_one.tile([128, FT, NH], BF, tag="w1")
            w2_sb = p_one.tile([128, NC, NC], BF, tag="w2")
            b2_sb = p_one.tile([128, NC], F32, tag="b2")
            vec_sb = p_one.tile([128, MT, 8], F32, tag="vecs")
            ident = p_one.tile([128, 128], BF, tag="ident")
            s_sb = p_one.tile([128, MT, NH], F32, tag="s")
            zq_sb = p_one.tile([128, MT, NH], F8, tag="zq")
            ztA_sb = p_one.tile([128, 4, NH], F8, tag="ztqA")
            ztB_sb = p_one.tile([128, 4, NH], F8, tag="ztqB")
            zb_c = [p_one.tile([128, KT // 8, NH], F8, tag=f"zb{j}",
                               name=f"zb{j}") for j in range(8)]
            zb2A_c = [p_one.tile([128, 8, NH], F8, tag=f"zb2A_{j}",
                                 name=f"zb2A_{j}") for j in range(4)]
            zb2B_c = [p_one.tile([128, 8, NH], F8, tag=f"zb2B_{j}",
                                 name=f"zb2B_{j}") for j in range(4)]
            zvfA = p_one.tile([128, KT // 2, NC], F8, tag="zvfA")
            zvfB = p_one.tile([128, KT // 2, NC], F8, tag="zvfB")
            zufA = p_one.tile([128, KT // 2, NC], F8, tag="zufA")
            zufB = p_one.tile([128, KT // 2, NC], F8, tag="zufB")
            zvA_sb = p_one.tile([128, 4, NC], F8, tag="zvsA")
            zvB_sb = p_one.tile([128, 4, NC], F8, tag="zvsB")
            zuA_sb = p_one.tile([128, 4, NC], F8, tag="zusA")
            zuB_sb = p_one.tile([128, 4, NC], F8, tag="zusB")
            vh_sb = p_one.tile([128, MT, NC], F32, tag="vh")
            usb = p_one.tile([128, MT, NC], F32, tag="usb")
            pre_sub = p_one.tile([128, MT, NC], F32, tag="presub")
            G_sb = p_one.tile([128, 4, NC], F32, tag="G")
            G2_sb = p_one.tile([128, 4, NC], F32, tag="G2")
            out_sb = p_one.tile([128, MT, NC], F32, tag="osb")
            hp_sb = p_one.tile([128, MT, NH], BF, tag="hp_sb")

            # pre-place the natural_log_exp_and_others table (set 6): it
            # contains exp, ln, copy and relu, so the act-table inserter
            # adds no further loads anywhere in the kernel
            nc.scalar.add_instruction(mybir.InstLoadActFuncSet(
                name=nc.get_next_instruction_name(), act_func_set_id=6,
                ins=[], outs=[]))

            # small static loads first (SP queue), then the slab stream
            nc.sync.dma_start(out=w1_sb, in_=w1[:].rearrange(
                "(kt p) n -> p kt n", p=128))
            for kt in range(FT):
                nc.sync.dma_start(out=xT_sb[:, kt, :],
                                  in_=xT[ts(kt, 128), :])
            nc.sync.dma_start(out=w2_sb, in_=w2h[:].rearrange(
                "(kt p) n -> p kt n", p=128))
            nc.sync.dma_start(out=b2_sb, in_=b2[:].to_broadcast([128, NC]))
            nc.sync.dma_start(out=vec_sb, in_=vecs[:].rearrange(
                "(mt p) j -> p mt j", p=128))
            masks.make_identity(nc, ident)

            adj_c = []
            for t2 in range(PT):
                a = p_adj.tile([128, 2, RPC], F8, tag="adj", name=f"adj{t2}")
                nc.sync.dma_start(
                    out=a,
                    in_=adjq[ts(t2, 256), :].rearrange(
                        "(k p) m -> p k m", p=128))
                adj_c.append(a)

            def adj_pair(t2, mt):
                # DoubleRow stationary slice for kt-pair t2, row block mt
                return adj_c[t2][:, :, ts(mt, 128)]

            with ExitStack() as c1:
                ps_s = c1.enter_context(
                    tc.tile_pool(name="ps_s", bufs=8, space="PSUM"))
                # ---- s = x @ W1 (kt-outer: chases the xT chunk DMAs);
                # quantize+scale rows for the gather ----
                psx = [ps_s.tile([128, NH], F32, tag="ps",
                                 name=f"psx{j}") for j in range(MT)]
                # warm the PE p-state before x@W1 (its latency feeds the
                # zs gather chain): self-contained junk groups into psx[0],
                # wiped by x@W1's start=True
                for i in range(2):
                    nc.tensor.matmul(psx[0], ident, w1_sb[:, 0, :],
                                     start=True, stop=True,
                                     skip_group_check=True)
                for kt in range(FT):
                    for mt in range(MT):
                        nc.tensor.matmul(psx[mt],
                                         xT_sb[:, kt, ts(mt, 128)],
                                         w1_sb[:, kt, :],
                                         start=kt == 0, stop=kt == FT - 1)
                for mt in range(MT):
                    nc.scalar.activation(out=s_sb[:, mt, :], in_=psx[mt],
                                         func=AF.Copy)
                    nc.vector.tensor_scalar(zq_sb[:, mt, :], s_sb[:, mt, :],
                                            vec_sb[:, mt, 1:2], None,
                                            op0=OP.mult)
                nc.scalar.dma_start(
                    out=zs_in[:].rearrange("(mt p) n -> p mt n", p=128),
                    in_=zq_sb)
                gather(zs_in, zs_out)
                for j in range(8):
                    eng = nc.scalar if j == 0 else nc.gpsimd
                    eng.dma_start(
                        out=zb_c[j],
                        in_=zs_out[j * (N // 8):(j + 1) * (N // 8), :]
                        .rearrange("(kt p) n -> p kt n", p=128))

            with ExitStack() as c2a:
                pm1 = c2a.enter_context(
                    tc.tile_pool(name="pm1", bufs=8, space="PSUM"))
                # ---- pass 1: psum1 = sig1 * P(s), one sweep, all mt ----
                ps1 = [pm1.tile([128, NH], F32, tag="pm1",
                                name=f"ps1_{j}") for j in range(MT)]
                for t2 in range(PT):
                    zb_ap = zb_c[t2 // 4][:, ts(t2 % 4, 2), :]
                    for mt in range(MT):
                        nc.tensor.matmul(
                            ps1[mt], adj_pair(t2, mt), zb_ap,
                            start=t2 == 0, stop=t2 == PT - 1, perf_mode=DR)
                def zt_half(h, zt_sbh, zt_inh, zt_outh, zb2h):
                    for j in range(4):
                        mt = 4 * h + j
                        # T = Dr*P1 ; zt = fp8(sig2*Dc*T) ; s <- s - T
                        T_t = p_rot.tile([128, NH], F32, tag="T", bufs=4)
                        nc.scalar.activation(out=T_t, in_=ps1[mt],
                                             func=AF.Copy,
                                             scale=vec_sb[:, mt, 0:1])
                        nc.vector.tensor_scalar(zt_sbh[:, j, :], T_t,
                                                vec_sb[:, mt, 2:3], None,
                                                op0=OP.mult)
                        nc.vector.tensor_sub(s_sb[:, mt, :], s_sb[:, mt, :],
                                             T_t)
                    for js in range(2):
                        nc.sync.dma_start(
                            out=zt_inh[js * 256:(js + 1) * 256, :]
                            .rearrange("(mt p) n -> p mt n", p=128),
                            in_=zt_sbh[:, 2 * js:2 * js + 2, :])
                    gather(zt_inh, zt_outh, eng=nc.sync)
                    nc.sync.dma_start(
                        out=zb2h[0][:, 0:2, :],
                        in_=zt_outh[0:256, :]
                        .rearrange("(ka p) n -> p ka n", p=128))
                    nc.sync.dma_start(
                        out=zb2h[0][:, 2:5, :],
                        in_=zt_outh[256:640, :]
                        .rearrange("(ka p) n -> p ka n", p=128))
                    nc.sync.dma_start(
                        out=zb2h[0][:, 5:8, :],
                        in_=zt_outh[640:RPC, :]
                        .rearrange("(ka p) n -> p ka n", p=128))
                    for j in range(1, 4):
                        nc.gpsimd.dma_start(
                            out=zb2h[j],
                            in_=zt_outh[j * RPC:(j + 1) * RPC, :]
                            .rearrange("(ka p) n -> p ka n", p=128))
                zt_half(0, ztA_sb, zt_inA, zt_outA, zb2A_c)
                zt_half(1, ztB_sb, zt_inB, zt_outB, zb2B_c)

            with ExitStack() as c2:
                pm = c2.enter_context(
                    tc.tile_pool(name="pm", bufs=4, space="PSUM"))
                ptr = c2.enter_context(
                    tc.tile_pool(name="ptr", bufs=2, space="PSUM"))
                pv = c2.enter_context(
                    tc.tile_pool(name="pv", bufs=2, space="PSUM"))

                # ---- pass 2: psum2 = sig2 * P(Dr*P1), 2 groups of 4 ----
                # pair t2 covers kt {2t2, 2t2+1}; q = t2 % 4 selects the
                # A half (q < 2: local rows 0-511 of core t2//4) or B half.
                # ka = 4*(t2//4) + (2q mod 4) indexes rows of the half
                # buffer; chunk = ka // 8, offset = ka % 8.
                def zb2_ap(t2):
                    i, q = t2 // 4, t2 % 4
                    if q < 2:
                        ka = 4 * i + 2 * q
                        return zb2A_c[ka // 8][:, ka % 8:ka % 8 + 2, :]
                    ka = 4 * i + 2 * (q - 2)
                    return zb2B_c[ka // 8][:, ka % 8:ka % 8 + 2, :]

                p2_order = ([t2 for t2 in range(PT) if t2 % 4 < 2]
                            + [t2 for t2 in range(PT) if t2 % 4 >= 2])
                for g in range(2):
                  mts = range(4 * g, 4 * g + 4)
                  ps2 = {mt: pm.tile([128, NH], F32, tag="pm",
                                     name=f"ps2_{mt}") for mt in mts}
                  for idx, t2 in enumerate(p2_order):
                    zb_ap = zb2_ap(t2)
                    for mt in mts:
                        nc.tensor.matmul(
                            ps2[mt], adj_pair(t2, mt), zb_ap,
                            start=idx == 0, stop=idx == PT - 1,
                            perf_mode=DR)
                  for mt in mts:
                    # h' = relu(s - T - 2*Dr*P2) = 2h ; v = h' @ (W2/2)
                      t_sc = p_rot.tile([128, NH], F32, tag="tsc", bufs=4)
                      nc.scalar.activation(out=t_sc, in_=ps2[mt],
                                           func=AF.Copy,
                                           scale=vec_sb[:, mt, 3:4])
                      nc.vector.tensor_sub(s_sb[:, mt, :], s_sb[:, mt, :],
                                           t_sc)
                      hp_t = p_rot.tile([128, NH], BF, tag="hp", bufs=4)
                      nc.vector.tensor_scalar_max(hp_t, s_sb[:, mt, :], 0.0)
                      psv = pv.tile([128, NC], F32, tag="pv")
                      for kh in range(2):
                          pstr = ptr.tile([128, 128], BF, tag="ptr")
                          nc.tensor.transpose(pstr, hp_t[:, ts(kh, 128)],
                                              ident)
                          hT_t = p_rot.tile([128, 128], BF, tag="hT", bufs=3)
                          if kh == 0:
                              nc.scalar.activation(out=hT_t, in_=pstr,
                                                   func=AF.Copy)
                          else:
                              nc.vector.tensor_copy(hT_t, pstr)
                          nc.tensor.matmul(psv, hT_t, w2_sb[:, kh, :],
                                           start=kh == 0, stop=kh == 1)
                      zv_sbh = zvA_sb if mt < 4 else zvB_sb
                      nc.vector.tensor_scalar(zv_sbh[:, mt % 4, :], psv,
                                              vec_sb[:, mt, 4:5], None,
                                              op0=OP.mult)
                      nc.vector.tensor_scalar_mul(vh_sb[:, mt, :], psv, 0.5)
                  # fire this half's zv gather chain right after its group
                  zv_sbh = zvA_sb if g == 0 else zvB_sb
                  zv_inh = zv_inA if g == 0 else zv_inB
                  zv_outh = zv_outA if g == 0 else zv_outB
                  zvfh = zvfA if g == 0 else zvfB
                  nc.sync.dma_start(
                      out=zv_inh[:].rearrange("(mt p) c -> p mt c", p=128),
                      in_=zv_sbh)
                  gather(zv_inh, zv_outh, eng=nc.sync)
                  for jc in range(2):
                      nc.sync.dma_start(
                          out=zvfh[:, jc * (KT // 4):(jc + 1) * (KT // 4), :],
                          in_=zv_outh[jc * (N // 4):(jc + 1) * (N // 4), :]
                          .rearrange("(ka p) c -> p ka c", p=128))

            # ---------- layer-2 narrow passes ----------
            with ExitStack() as c3:
                puw = c3.enter_context(
                    tc.tile_pool(name="puw", bufs=8, space="PSUM"))

                def half_ap(zfA, zfB, t2):
                    i, q = t2 // 4, t2 % 4
                    if q < 2:
                        ka = 4 * i + 2 * q
                        return zfA[:, ka:ka + 2, :]
                    ka = 4 * i + 2 * (q - 2)
                    return zfB[:, ka:ka + 2, :]

                p3_order = ([t2 for t2 in range(PT) if t2 % 4 < 2]
                            + [t2 for t2 in range(PT) if t2 % 4 >= 2])
                # pass 3: psum3 = sig3 * u' ; u' = P(v)
                ps3 = {mt: puw.tile([128, NC], F32, tag="pu",
                                    name=f"ps3_{mt}") for mt in range(MT)}
                for idx, t2 in enumerate(p3_order):
                    for mt in range(MT):
                        nc.tensor.matmul(
                            ps3[mt], adj_pair(t2, mt),
                            half_ap(zvfA, zvfB, t2),
                            start=idx == 0, stop=idx == PT - 1,
                            perf_mode=DR)
                for h, (zu_sbh, zu_inh, zu_outh, zufh) in enumerate([
                        (zuA_sb, zu_inA, zu_outA, zufA),
                        (zuB_sb, zu_inB, zu_outB, zufB)]):
                    for j in range(4):
                        mt = 4 * h + j
                        nc.vector.tensor_scalar(zu_sbh[:, j, :], ps3[mt],
                                                vec_sb[:, mt, 5:6], None,
                                                op0=OP.mult)
                        nc.scalar.activation(out=usb[:, mt, :], in_=ps3[mt],
                                             func=AF.Copy, scale=0.5 / SIG3)
                    nc.sync.dma_start(
                        out=zu_inh[:].rearrange("(mt p) c -> p mt c",
                                                p=128),
                        in_=zu_sbh)
                    gather(zu_inh, zu_outh, eng=nc.sync)
                    for jc in range(2):
                        nc.sync.dma_start(
                            out=zufh[:, jc * (KT // 4):(jc + 1) * (KT // 4),
                                     :],
                            in_=zu_outh[jc * (N // 4):(jc + 1) * (N // 4), :]
                            .rearrange("(ka p) c -> p ka c", p=128))

                # pass 4: psum4 = sig4 * w' ; w' = P(Dr*u')
                ps4 = {mt: puw.tile([128, NC], F32, tag="pu",
                                    name=f"ps4_{mt}") for mt in range(MT)}
                for idx, t2 in enumerate(p3_order):
                    for mt in range(MT):
                        nc.tensor.matmul(
                            ps4[mt], adj_pair(t2, mt),
                            half_ap(zufA, zufB, t2),
                            start=idx == 0, stop=idx == PT - 1,
                            perf_mode=DR)

                # pre_sub = Dr*0.5u' - b2, overlapped with pass-4 matmuls
                for mt in range(MT):
                    nc.vector.tensor_scalar(pre_sub[:, mt, :], usb[:, mt, :],
                                            vec_sb[:, mt, 6:7], None,
                                            op0=OP.mult)
                    nc.vector.tensor_sub(pre_sub[:, mt, :],
                                         pre_sub[:, mt, :], b2_sb)
                # G = vh - (Dr/sig4)*psum4 - pre_sub  (= out_pre + b2)
                G_h = [G_sb, G2_sb]
                for mt in range(MT):
                    g_t = G_h[mt // 4]
                    gj = mt % 4
                    tmp = p_rot.tile([128, NC], F32, tag="tmp", bufs=8)
                    nc.vector.tensor_scalar(tmp, ps4[mt],
                                            vec_sb[:, mt, 7:8], None,
                                            op0=OP.mult)
                    nc.vector.tensor_sub(tmp, vh_sb[:, mt, :], tmp)
                    nc.vector.tensor_sub(g_t[:, gj, :], tmp,
                                         pre_sub[:, mt, :])
                # log_softmax (2 classes, |o| small so no max-shift):
                # out = o - ln(exp(o0) + exp(o1))
                ex_t = p_rot.tile([128, 4, NC], F32, tag="ex", name="ex0")
                ex2_t = p_rot.tile([128, 4, NC], F32, tag="ex2", name="ex1")
                nc.scalar.activation(out=ex_t, in_=G_sb, func=AF.Exp)
                nc.scalar.activation(out=ex2_t, in_=G2_sb, func=AF.Exp)
                sm_t = p_rot.tile([128, 4], F32, tag="sm", name="sm0")
                sm2_t = p_rot.tile([128, 4], F32, tag="sm2", name="sm1")
                nc.vector.tensor_add(sm_t, ex_t[:, :, 0], ex_t[:, :, 1])
                nc.vector.tensor_add(sm2_t, ex2_t[:, :, 0], ex2_t[:, :, 1])
                lg_t = p_rot.tile([128, 4], F32, tag="lg", name="lg0")
                lg2_t = p_rot.tile([128, 4], F32, tag="lg2", name="lg1")
                nc.scalar.activation(out=lg_t, in_=sm_t, func=AF.Ln)
                nc.scalar.activation(out=lg2_t, in_=sm2_t, func=AF.Ln)
                lgs = [lg_t, lg2_t]
                out_hs = [p_rot.tile([128, 4, NC], F32, tag=f"oh{h}",
                                     name=f"oh{h}") for h in range(2)]
                for h in range(2):
                    for gj in range(4):
                        mt = 4 * h + gj
                        nc.vector.tensor_scalar(out_hs[h][:, gj, :],
                                                G_h[h][:, gj, :],
                                                lgs[h][:, gj:gj + 1],
                                                None, op0=OP.subtract)
                    nc.sync.dma_start(
                        out=out[h * RH:(h + 1) * RH, :].rearrange(
                            "(mt p) c -> p mt c", p=128),
                        in_=out_hs[h])

    nc.compile()
    return nc


def _get_nc(lite=False):
    key = "nc_lite" if lite else "nc"
    if key not in _CACHE:
        _CACHE[key] = _build(lite=lite)
    return _CACHE[key]


def _prep_in_maps(x, adj, W1, W2, b2):
    bf = ml_dtypes.bfloat16
    f8 = ml_dtypes.float8_e4m3fn
    f32 = np.float32
    x = np.asarray(x, f32)
    adj = np.asarray(adj, f32)
    w1 = np.asarray(W1, f32).astype(bf)
    w2h = (0.5 * np.asarray(W2, f32)).astype(bf)
    b2v = np.asarray(b2, f32).reshape(1, NC)

    d_row = adj.sum(1) ** -0.5
    d_row[np.isinf(d_row)] = 0.0
    d_col = adj.sum(0) ** -0.5
    d_col[np.isinf(d_col)] = 0.0
    d_row = d_row.astype(f32)
    d_col = d_col.astype(f32)

    in_maps = []
    for i in range(NCORE):
        rows = slice(i * RPC, (i + 1) * RPC)
        drL = d_row[rows]
        dcL = d_col[rows]
        vecs = np.zeros((RPC, 8), f32)
        vecs[:, 0] = drL / SIG1                  # T scale
        vecs[:, 1] = SIG1 * dcL                  # zs quantize
        vecs[:, 2] = SIG2 * dcL                  # zt quantize (from T)
        vecs[:, 3] = 2.0 * drL / SIG2            # pass-2 scale
        vecs[:, 4] = SIG3 * dcL                  # zv quantize (from psv)
        vecs[:, 5] = (SIG4 / SIG3) * dcL * drL   # zu quantize (from psum3)
        vecs[:, 6] = drL                         # pre_sub Dr
        vecs[:, 7] = drL / SIG4                  # pass-4 G scale
        in_maps.append({
            "adjq": adj[rows, :].T.astype(f8),   # fp8 slab, one fused cast
            "xT": x[rows, :].T.astype(bf),
            "w1": w1, "w2h": w2h, "b2": b2v, "vecs": vecs,
        })
    return in_maps


def _run(x, adj, W1, W2, b2, trace=False, lite=False, in_maps=None):
    from concourse.bass_utils import run_bass_kernel_spmd
    nc = _get_nc(lite=lite)
    if in_maps is None:
        in_maps = _prep_in_maps(x, adj, W1, W2, b2)
    res = run_bass_kernel_spmd(nc, in_maps, core_ids=list(range(NCORE)),
                               trace=trace)
    out = np.concatenate([r["out"] for r in res.results], axis=0)
    return out, res


def kernel(x, adj, W1, W2, b2):
    out, _ = _run(x, adj, W1, W2, b2, trace=False)
    return out
